# revision 5
# baseline (speedup 1.0000x reference)
"""Trainium2 Bass kernel for the edge-MLP decoder (gnn_message_passing), v2.

Computes, for every edge (s, d):
    out = sigmoid(relu(relu([z[s]; z[d]] @ W1 + b1) @ W2 + b2) @ W3 + b3)

v2 strategy (vs the HBM-gather baseline):
  * Node ids are split into 4 ranges of 25000. Edges are bucketed into 16
    (src_range, dst_range) classes; core c owns classes 2c and 2c+1 (which
    share a src range), so each core needs at most 3 of the 4 z-ranges.
  * Those <=3 ranges are held RESIDENT in SBUF as token tables (19.2 MB in
    fp16), and every per-edge z-row fetch is an SBUF-source dma_gather
    (transpose mode) instead of a random 256B HBM read - sidestepping the
    sub-512B HBM descriptor penalty and all HBM random-read inefficiency.
  * Gathers and idx streaming are chunked (2048 edges) and double-buffered
    so SWDGE descriptor generation, SDMA transfer and PE compute pipeline.
  * Matmuls run in fp16 (full PE rate), fp32 PSUM accumulation; bias+relu
    on ACT and DVE; the W3 dot rides a PSUM-accumulation trick (one matmul
    per 512-edge block, one sigmoid per 128 blocks).
"""

import numpy as np
from contextlib import ExitStack

import concourse.bass as bass
import concourse.tile as tile
from concourse import bacc, mybir
from concourse.bass_utils import run_bass_kernel_spmd

# ---- static problem geometry (nn_Decoder_81819126989051) ----
N_NODES = 100000
D = 128                   # node feature dim
N_CORES = 8
RANGE = 25000             # node-id range per table (int16-safe)
NRANGE = N_NODES // RANGE  # 4
NCLS = NRANGE * NRANGE    # 16 classes; core c owns classes 2c, 2c+1
NTOK = 25088              # table tokens (196 ranks x 128 partitions)
NRANK = NTOK // 128       # 196
BLK = 512                 # edges per matmul sub-block (PSUM bank width)
CHUNK = 2048              # edges per gather call (4 blocks)
CAP_SLOT = 63488          # edge slots per class (max class ~63090)
NCHUNK = CAP_SLOT // CHUNK  # chunks per class slot
SCRATCH = 16384           # SWDGE descriptor-ring carveout bytes/partition
B_SLOT = CAP_SLOT // BLK  # 124 blocks per slot
B_TOT = 2 * B_SLOT        # 248 blocks per core
IDXC = CAP_SLOT // 16     # 3968 idx columns per slot (wrapped int16)
OUT_CH = (B_TOT + 127) // 128  # 2 output staging column chunks

F16 = mybir.dt.float16
F32 = mybir.dt.float32
I16 = mybir.dt.int16
AF = mybir.ActivationFunctionType
ALU = mybir.AluOpType

_prog_cache = None
QUEUES = 1  # SWDGE queues: >1 corrupts (concurrent xbar transpose streams)


def _build_program(do_gather=True, do_compute=True, reps=1, queues=None):
    if queues is None:
        queues = QUEUES
    nc = bacc.Bacc(
        "TRN2", target_bir_lowering=False, debug=False, num_devices=N_CORES,
        dynamic_dma_scratch_size=SCRATCH, num_swdge_queues=queues,
    )

    tabS_d = nc.declare_dram_parameter("tabS", [128, NTOK], F16, isOutput=False)
    tabD0_d = nc.declare_dram_parameter("tabD0", [128, NTOK], F16, isOutput=False)
    tabD1_d = nc.declare_dram_parameter("tabD1", [128, NTOK], F16, isOutput=False)
    sidx_d = [nc.declare_dram_parameter(f"sidx{s}", [128, IDXC], I16, isOutput=False)
              for s in range(2)]
    didx_d = [nc.declare_dram_parameter(f"didx{s}", [128, IDXC], I16, isOutput=False)
              for s in range(2)]
    w1s_d = nc.declare_dram_parameter("w1s", [128, 256], F16, isOutput=False)
    w1d_d = nc.declare_dram_parameter("w1d", [128, 256], F16, isOutput=False)
    w2a_d = nc.declare_dram_parameter("w2a", [128, 128], F16, isOutput=False)
    w2b_d = nc.declare_dram_parameter("w2b", [128, 128], F16, isOutput=False)
    # w3v[:, 127] = W3; all other columns zero.  lhsT slice [127-p : 255-p]
    # puts W3 in output-partition p of the shared logit PSUM bank, so 128
    # blocks accumulate into one [128, 512] tile -> one sigmoid per chunk.
    w3v_d = nc.declare_dram_parameter("w3v", [128, 255], F16, isOutput=False)
    b1a_d = nc.declare_dram_parameter("b1a", [128, 1], F32, isOutput=False)
    b1b_d = nc.declare_dram_parameter("b1b", [128, 1], F32, isOutput=False)
    b2_d = nc.declare_dram_parameter("b2", [128, 1], F32, isOutput=False)
    b3_d = nc.declare_dram_parameter("b3", [128, 1], F32, isOutput=False)
    out_d = nc.declare_dram_parameter("out", [B_TOT, BLK], F32, isOutput=True)

    with tile.TileContext(nc) as tc, ExitStack() as ctx:
        const = ctx.enter_context(tc.tile_pool(name="const", bufs=1))

        def make_const(dram, shape, dtype):
            t = const.tile(shape, dtype, tag=dram.name + "_sb",
                           name=dram.name + "_sb")
            return t

        def load_const(t, dram):
            nc.sync.dma_start(out=t[:], in_=dram[:])

        tw1s = make_const(w1s_d, [128, 256], F16)
        tw1d = make_const(w1d_d, [128, 256], F16)
        tw2a = make_const(w2a_d, [128, 128], F16)
        tw2b = make_const(w2b_d, [128, 128], F16)
        tw3v = make_const(w3v_d, [128, 255], F16)
        tb1a = make_const(b1a_d, [128, 1], F32)
        tb1b = make_const(b1b_d, [128, 1], F32)
        tb2 = make_const(b2_d, [128, 1], F32)
        tb3 = make_const(b3_d, [128, 1], F32)
        ttabS = make_const(tabS_d, [128, NTOK], F16)
        ttabD0 = make_const(tabD0_d, [128, NTOK], F16)
        ttabD1 = make_const(tabD1_d, [128, NTOK], F16)
        tout = const.tile([128, OUT_CH * BLK], F32, tag="out_sb")

        ipool = ctx.enter_context(tc.tile_pool(name="idx", bufs=4))
        gpool = ctx.enter_context(tc.tile_pool(name="gath", bufs=4))
        h1pool = ctx.enter_context(tc.tile_pool(name="h1s", bufs=4))
        h2pool = ctx.enter_context(tc.tile_pool(name="h2s", bufs=3))
        ph1 = ctx.enter_context(tc.tile_pool(name="ph1", bufs=4, space="PSUM"))
        ph2 = ctx.enter_context(tc.tile_pool(name="ph2", bufs=2, space="PSUM"))
        plg = ctx.enter_context(tc.tile_pool(name="plg", bufs=2, space="PSUM"))

        for _rep in range(reps):
            _emit_workload(
                nc, do_gather, do_compute, load_const, tw1s, tw1d, tw2a, tw2b,
                tw3v, tb1a, tb1b, tb2, tb3, ttabS, ttabD0, ttabD1, tout,
                ipool, gpool, h1pool, h2pool, ph1, ph2, plg,
                w1s_d, w1d_d, w2a_d, w2b_d, w3v_d, b1a_d, b1b_d, b2_d, b3_d,
                tabS_d, tabD0_d, tabD1_d, sidx_d, didx_d, out_d, queues,
            )

    nc.compile()
    return nc


def _emit_workload(nc, do_gather, do_compute, load_const, tw1s, tw1d, tw2a,
                   tw2b, tw3v, tb1a, tb1b, tb2, tb3, ttabS, ttabD0, ttabD1,
                   tout, ipool, gpool, h1pool, h2pool, ph1, ph2, plg,
                   w1s_d, w1d_d, w2a_d, w2b_d, w3v_d, b1a_d, b1b_d, b2_d,
                   b3_d, tabS_d, tabD0_d, tabD1_d, sidx_d, didx_d, out_d,
                   queues=1):
        load_const(tw1s, w1s_d)
        load_const(tw1d, w1d_d)
        load_const(tw2a, w2a_d)
        load_const(tw2b, w2b_d)
        load_const(tw3v, w3v_d)
        load_const(tb1a, b1a_d)
        load_const(tb1b, b1b_d)
        load_const(tb2, b2_d)
        load_const(tb3, b3_d)
        load_const(ttabS, tabS_d)
        load_const(ttabD0, tabD0_d)
        load_const(ttabD1, tabD1_d)

        lg = None
        last_b = B_TOT - 1
        if not do_gather:
            # compute-only ablation: all blocks read one preinitialized tile
            dummy = gpool.tile([128, 1, CHUNK], F16, tag="gath")
            nc.vector.memset(dummy[:], 0.25)
        qn = 0
        icols = CHUNK // 16  # 128
        for s in range(2):
            tabD = ttabD0 if s == 0 else ttabD1
            # per-slot idx preload: keeps the gather stream free of
            # interleaved small DMAs
            tsix = ipool.tile([128, IDXC], I16, tag="sidx", bufs=1)
            nc.sync.dma_start(out=tsix[:], in_=sidx_d[s][:])
            tdix = ipool.tile([128, IDXC], I16, tag="didx", bufs=1)
            nc.sync.dma_start(out=tdix[:], in_=didx_d[s][:])
            for c in range(NCHUNK):
                if do_gather:
                    sg = gpool.tile([128, 1, CHUNK], F16, tag="gath")
                    dg = gpool.tile([128, 1, CHUNK], F16, tag="gath")
                    nc.gpsimd.dma_gather(
                        sg[:], ttabS[:], tsix[:, c * icols:(c + 1) * icols],
                        CHUNK, CHUNK, D,
                        transpose=True, single_packet=False,
                        sbuf_tokens_per_rank=128, sbuf_free_dim_per_rank=256,
                        queue_num=qn,
                    )
                    qn = (qn + 1) % queues
                    nc.gpsimd.dma_gather(
                        dg[:], tabD[:], tdix[:, c * icols:(c + 1) * icols],
                        CHUNK, CHUNK, D,
                        transpose=True, single_packet=False,
                        sbuf_tokens_per_rank=128, sbuf_free_dim_per_rank=256,
                        queue_num=qn,
                    )
                    qn = (qn + 1) % queues
                else:
                    sg = dg = dummy
                if not do_compute:
                    continue
                for j in range(CHUNK // BLK):
                    b = s * B_SLOT + c * (CHUNK // BLK) + j
                    sT = sg[:, 0, j * BLK:(j + 1) * BLK]
                    dT = dg[:, 0, j * BLK:(j + 1) * BLK]

                    h1a = ph1.tile([128, BLK], F32, tag="ph1")
                    nc.tensor.matmul(out=h1a[:], lhsT=tw1s[:, 0:128], rhs=sT, start=True, stop=False)
                    nc.tensor.matmul(out=h1a[:], lhsT=tw1d[:, 0:128], rhs=dT, start=False, stop=True)
                    h1b = ph1.tile([128, BLK], F32, tag="ph1")
                    nc.tensor.matmul(out=h1b[:], lhsT=tw1s[:, 128:256], rhs=sT, start=True, stop=False)
                    nc.tensor.matmul(out=h1b[:], lhsT=tw1d[:, 128:256], rhs=dT, start=False, stop=True)

                    h1sa = h1pool.tile([128, BLK], F16, tag="h1s")
                    nc.scalar.activation(h1sa[:], h1a[:], AF.Relu, bias=tb1a[:])
                    h1sb = h1pool.tile([128, BLK], F16, tag="h1s")
                    nc.vector.tensor_scalar(
                        out=h1sb[:], in0=h1b[:], scalar1=tb1b[:], scalar2=0.0,
                        op0=ALU.add, op1=ALU.max,
                    )

                    h2p = ph2.tile([128, BLK], F32, tag="ph2")
                    nc.tensor.matmul(out=h2p[:], lhsT=tw2a[:], rhs=h1sa[:], start=True, stop=False)
                    nc.tensor.matmul(out=h2p[:], lhsT=tw2b[:], rhs=h1sb[:], start=False, stop=True)
                    h2s = h2pool.tile([128, BLK], F16, tag="h2s")
                    nc.vector.tensor_scalar(
                        out=h2s[:], in0=h2p[:], scalar1=tb2[:], scalar2=0.0,
                        op0=ALU.add, op1=ALU.max,
                    )

                    p, ch = b % 128, b // 128
                    if p == 0:
                        lg = plg.tile([128, BLK], F32, tag="plg")
                    nc.tensor.matmul(
                        out=lg[:], lhsT=tw3v[:, 127 - p:255 - p], rhs=h2s[:],
                        start=(p == 0), stop=(p == 127 or b == last_b),
                        skip_group_check=True,
                    )
                    if p == 127 or b == last_b:
                        nc.scalar.activation(
                            tout[:, ch * BLK:(ch + 1) * BLK], lg[:], AF.Sigmoid,
                            bias=tb3[:],
                        )

        if do_compute:
            for ch in range(OUT_CH):
                rows = min(128, B_TOT - ch * 128)
                nc.sync.dma_start(
                    out=out_d[ch * 128: ch * 128 + rows, :],
                    in_=tout[0:rows, ch * BLK:(ch + 1) * BLK],
                )


def _w3v(W3):
    v = np.zeros((128, 255), np.float16)
    v[:, 127] = W3.astype(np.float16).reshape(-1)
    return v


def _wrap_idx(arr):
    """[CAP_SLOT] int16 -> [128, IDXC] wrapped (16-partition, replicated x8)."""
    t = arr.reshape(-1, 16).T  # [16, IDXC]
    return np.tile(t, (8, 1))


def _pack_table(z16_part):
    """[<=25000, 128] fp16 -> [128, NTOK] token table (token t: partition
    t%128, rank t//128)."""
    pad = np.zeros((NTOK, D), np.float16)
    pad[: z16_part.shape[0]] = z16_part
    return np.ascontiguousarray(
        pad.reshape(NRANK, 128, D).transpose(1, 0, 2).reshape(128, NTOK)
    )


def _mlp_ref_f32(zs, zd, W1, b1, W2, b2, W3, b3):
    ef = np.concatenate([zs, zd], axis=1)
    h = np.maximum(ef @ W1 + b1, 0.0)
    h = np.maximum(h @ W2 + b2, 0.0)
    o = h @ W3 + b3
    return 1.0 / (1.0 + np.exp(-o[:, 0]))


def _pack_inputs(z, ei, W1, b1, W2, b2, W3, b3):
    """Class-bucket edges across cores; returns (in_maps, metas)."""
    src = ei[0].astype(np.int64)
    dst = ei[1].astype(np.int64)
    z16 = z.astype(np.float16)
    tables = [_pack_table(z16[r * RANGE:(r + 1) * RANGE]) for r in range(NRANGE)]
    w_common = {
        "w1s": np.ascontiguousarray(W1[:128].astype(np.float16)),
        "w1d": np.ascontiguousarray(W1[128:].astype(np.float16)),
        "w2a": np.ascontiguousarray(W2[:128].astype(np.float16)),
        "w2b": np.ascontiguousarray(W2[128:].astype(np.float16)),
        "w3v": _w3v(W3),
        "b1a": np.ascontiguousarray(b1[:128].reshape(128, 1)),
        "b1b": np.ascontiguousarray(b1[128:].reshape(128, 1)),
        "b2": np.ascontiguousarray(b2.reshape(128, 1)),
        "b3": np.full((128, 1), np.float32(b3.reshape(-1)[0])),
    }

    cls = (src // RANGE) * NRANGE + (dst // RANGE)
    order = np.argsort(cls, kind="stable")
    counts = np.bincount(cls, minlength=NCLS)
    starts = np.zeros(NCLS + 1, np.int64)
    np.cumsum(counts, out=starts[1:])

    in_maps = []
    metas = []  # per core: (positions per slot, overflow positions)
    for c in range(N_CORES):
        rs = c // 2
        rd0 = (2 * c) % NRANGE
        rd1 = rd0 + 1
        in_map = {
            **w_common,
            "tabS": tables[rs],
            "tabD0": tables[rd0],
            "tabD1": tables[rd1],
        }
        kept = []
        overflow = []
        for s in range(2):
            k = 2 * c + s
            seg = order[starts[k]:starts[k + 1]]
            if len(seg) > CAP_SLOT:
                overflow.append(seg[CAP_SLOT:])
                seg = seg[:CAP_SLOT]
            n = len(seg)
            sidx = np.zeros(CAP_SLOT, np.int16)
            didx = np.zeros(CAP_SLOT, np.int16)
            sidx[:n] = (src[seg] - rs * RANGE).astype(np.int16)
            didx[:n] = (dst[seg] - (rd0 if s == 0 else rd1) * RANGE).astype(np.int16)
            in_map[f"sidx{s}"] = np.ascontiguousarray(_wrap_idx(sidx))
            in_map[f"didx{s}"] = np.ascontiguousarray(_wrap_idx(didx))
            kept.append(seg)
        metas.append((kept, overflow))
        in_maps.append(in_map)
    return in_maps, metas


def _unpack_outputs(core_outs, metas, ei, z, W1, b1, W2, b2, W3, b3):
    E = ei.shape[1]
    out = np.empty(E, dtype=np.float32)
    for c in range(N_CORES):
        flat = np.asarray(core_outs[c], dtype=np.float32).reshape(2, CAP_SLOT)
        kept, overflow = metas[c]
        for s in range(2):
            seg = kept[s]
            out[seg] = flat[s, : len(seg)]
        for seg in overflow:
            # Host fallback for edges beyond the static per-class capacity
            # (does not trigger for the benchmark dataset).
            out[seg] = _mlp_ref_f32(
                z[ei[0, seg]], z[ei[1, seg]], W1, b1, W2, b2, W3, b3)
    return out


def _run(z, edge_index, W1, b1, W2, b2, W3, b3, **spmd_kwargs):
    global _prog_cache
    z = np.asarray(z, dtype=np.float32)
    W1 = np.asarray(W1, dtype=np.float32)
    b1 = np.asarray(b1, dtype=np.float32)
    W2 = np.asarray(W2, dtype=np.float32)
    b2 = np.asarray(b2, dtype=np.float32)
    W3 = np.asarray(W3, dtype=np.float32)
    b3 = np.asarray(b3, dtype=np.float32)
    ei = np.asarray(edge_index).astype(np.int64)
    assert z.shape == (N_NODES, D) and ei.shape[0] == 2

    if _prog_cache is None:
        _prog_cache = _build_program()
    nc = _prog_cache

    in_maps, metas = _pack_inputs(z, ei, W1, b1, W2, b2, W3, b3)
    br = run_bass_kernel_spmd(nc, in_maps, list(range(N_CORES)), **spmd_kwargs)
    core_outs = [br.results[c]["out"] for c in range(N_CORES)]
    out = _unpack_outputs(core_outs, metas, ei, z, W1, b1, W2, b2, W3, b3)
    return out, br


def kernel(z, edge_index, W1, b1, W2, b2, W3, b3):
    out, _ = _run(z, edge_index, W1, b1, W2, b2, W3, b3)
    return out


# revision 6
# speedup vs baseline: 1.0761x; 1.0761x over previous
"""Trainium2 Bass kernel for the edge-MLP decoder (gnn_message_passing), v2.

Computes, for every edge (s, d):
    out = sigmoid(relu(relu([z[s]; z[d]] @ W1 + b1) @ W2 + b2) @ W3 + b3)

v2 strategy (vs the HBM-gather baseline):
  * Node ids are split into 4 ranges of 25000. Edges are bucketed into 16
    (src_range, dst_range) classes; core c owns classes 2c and 2c+1 (which
    share a src range), so each core needs at most 3 of the 4 z-ranges.
  * Those <=3 ranges are held RESIDENT in SBUF as token tables (19.2 MB in
    fp16), and every per-edge z-row fetch is an SBUF-source dma_gather
    (transpose mode) instead of a random 256B HBM read - sidestepping the
    sub-512B HBM descriptor penalty and all HBM random-read inefficiency.
  * Gathers and idx streaming are chunked (2048 edges) and double-buffered
    so SWDGE descriptor generation, SDMA transfer and PE compute pipeline.
  * Matmuls run in fp16 (full PE rate), fp32 PSUM accumulation; bias+relu
    on ACT and DVE; the W3 dot rides a PSUM-accumulation trick (one matmul
    per 512-edge block, one sigmoid per 128 blocks).
"""

import numpy as np
from contextlib import ExitStack

import concourse.bass as bass
import concourse.tile as tile
from concourse import bacc, mybir
from concourse.bass_utils import run_bass_kernel_spmd

# ---- static problem geometry (nn_Decoder_81819126989051) ----
N_NODES = 100000
D = 128                   # node feature dim
N_CORES = 8
RANGE = 25000             # node-id range per table (int16-safe)
NRANGE = N_NODES // RANGE  # 4
NCLS = NRANGE * NRANGE    # 16 classes; core c owns classes 2c, 2c+1
NTOK = 25088              # table tokens (196 ranks x 128 partitions)
NRANK = NTOK // 128       # 196
BLK = 512                 # edges per matmul sub-block (PSUM bank width)
CHUNK = 4096              # edges per gather call (8 blocks)
CAP_SLOT = 65536          # edge slots per class (max class ~63090)
NCHUNK = CAP_SLOT // CHUNK  # chunks per class slot
SCRATCH = 16384           # SWDGE descriptor-ring carveout bytes/partition
B_SLOT = CAP_SLOT // BLK  # 124 blocks per slot
B_TOT = 2 * B_SLOT        # 248 blocks per core
IDXC = CAP_SLOT // 16     # 3968 idx columns per slot (wrapped int16)
OUT_CH = (B_TOT + 127) // 128  # 2 output staging column chunks

F16 = mybir.dt.float16
F32 = mybir.dt.float32
I16 = mybir.dt.int16
AF = mybir.ActivationFunctionType
ALU = mybir.AluOpType

_prog_cache = None
QUEUES = 1  # SWDGE queues: >1 corrupts (concurrent xbar transpose streams)


def _build_program(do_gather=True, do_compute=True, reps=1, queues=None):
    if queues is None:
        queues = QUEUES
    nc = bacc.Bacc(
        "TRN2", target_bir_lowering=False, debug=False, num_devices=N_CORES,
        dynamic_dma_scratch_size=SCRATCH, num_swdge_queues=queues,
    )

    tabS_d = nc.declare_dram_parameter("tabS", [128, NTOK], F16, isOutput=False)
    tabD0_d = nc.declare_dram_parameter("tabD0", [128, NTOK], F16, isOutput=False)
    tabD1_d = nc.declare_dram_parameter("tabD1", [128, NTOK], F16, isOutput=False)
    sidx_d = [nc.declare_dram_parameter(f"sidx{s}", [128, IDXC], I16, isOutput=False)
              for s in range(2)]
    didx_d = [nc.declare_dram_parameter(f"didx{s}", [128, IDXC], I16, isOutput=False)
              for s in range(2)]
    w1s_d = nc.declare_dram_parameter("w1s", [128, 256], F16, isOutput=False)
    w1d_d = nc.declare_dram_parameter("w1d", [128, 256], F16, isOutput=False)
    w2a_d = nc.declare_dram_parameter("w2a", [128, 128], F16, isOutput=False)
    w2b_d = nc.declare_dram_parameter("w2b", [128, 128], F16, isOutput=False)
    # w3v[:, 127] = W3; all other columns zero.  lhsT slice [127-p : 255-p]
    # puts W3 in output-partition p of the shared logit PSUM bank, so 128
    # blocks accumulate into one [128, 512] tile -> one sigmoid per chunk.
    w3v_d = nc.declare_dram_parameter("w3v", [128, 255], F16, isOutput=False)
    b1a_d = nc.declare_dram_parameter("b1a", [128, 1], F32, isOutput=False)
    b1b_d = nc.declare_dram_parameter("b1b", [128, 1], F32, isOutput=False)
    b2_d = nc.declare_dram_parameter("b2", [128, 1], F32, isOutput=False)
    b3_d = nc.declare_dram_parameter("b3", [128, 1], F32, isOutput=False)
    out_d = nc.declare_dram_parameter("out", [B_TOT, BLK], F32, isOutput=True)

    with tile.TileContext(nc) as tc, ExitStack() as ctx:
        const = ctx.enter_context(tc.tile_pool(name="const", bufs=1))

        def make_const(dram, shape, dtype):
            t = const.tile(shape, dtype, tag=dram.name + "_sb",
                           name=dram.name + "_sb")
            return t

        def load_const(t, dram):
            nc.sync.dma_start(out=t[:], in_=dram[:])

        tw1s = make_const(w1s_d, [128, 256], F16)
        tw1d = make_const(w1d_d, [128, 256], F16)
        tw2a = make_const(w2a_d, [128, 128], F16)
        tw2b = make_const(w2b_d, [128, 128], F16)
        tw3v = make_const(w3v_d, [128, 255], F16)
        tb1a = make_const(b1a_d, [128, 1], F32)
        tb1b = make_const(b1b_d, [128, 1], F32)
        tb2 = make_const(b2_d, [128, 1], F32)
        tb3 = make_const(b3_d, [128, 1], F32)
        ttabS = make_const(tabS_d, [128, NTOK], F16)
        ttabD0 = make_const(tabD0_d, [128, NTOK], F16)
        ttabD1 = make_const(tabD1_d, [128, NTOK], F16)
        tout = const.tile([128, OUT_CH * BLK], F32, tag="out_sb")

        ipool = ctx.enter_context(tc.tile_pool(name="idx", bufs=4))
        gpool = ctx.enter_context(tc.tile_pool(name="gath", bufs=3))
        h1pool = ctx.enter_context(tc.tile_pool(name="h1s", bufs=4))
        h2pool = ctx.enter_context(tc.tile_pool(name="h2s", bufs=3))
        ph1 = ctx.enter_context(tc.tile_pool(name="ph1", bufs=4, space="PSUM"))
        ph2 = ctx.enter_context(tc.tile_pool(name="ph2", bufs=2, space="PSUM"))
        plg = ctx.enter_context(tc.tile_pool(name="plg", bufs=2, space="PSUM"))

        for _rep in range(reps):
            _emit_workload(
                nc, do_gather, do_compute, load_const, tw1s, tw1d, tw2a, tw2b,
                tw3v, tb1a, tb1b, tb2, tb3, ttabS, ttabD0, ttabD1, tout,
                ipool, gpool, h1pool, h2pool, ph1, ph2, plg,
                w1s_d, w1d_d, w2a_d, w2b_d, w3v_d, b1a_d, b1b_d, b2_d, b3_d,
                tabS_d, tabD0_d, tabD1_d, sidx_d, didx_d, out_d, queues,
            )

    nc.compile()
    return nc


def _emit_workload(nc, do_gather, do_compute, load_const, tw1s, tw1d, tw2a,
                   tw2b, tw3v, tb1a, tb1b, tb2, tb3, ttabS, ttabD0, ttabD1,
                   tout, ipool, gpool, h1pool, h2pool, ph1, ph2, plg,
                   w1s_d, w1d_d, w2a_d, w2b_d, w3v_d, b1a_d, b1b_d, b2_d,
                   b3_d, tabS_d, tabD0_d, tabD1_d, sidx_d, didx_d, out_d,
                   queues=1):
        load_const(tw1s, w1s_d)
        load_const(tw1d, w1d_d)
        load_const(tw2a, w2a_d)
        load_const(tw2b, w2b_d)
        load_const(tw3v, w3v_d)
        load_const(tb1a, b1a_d)
        load_const(tb1b, b1b_d)
        load_const(tb2, b2_d)
        load_const(tb3, b3_d)
        load_const(ttabS, tabS_d)
        load_const(ttabD0, tabD0_d)
        load_const(ttabD1, tabD1_d)

        lg = None
        last_b = B_TOT - 1
        if not do_gather:
            # compute-only ablation: all blocks read one preinitialized tile
            dummy = gpool.tile([128, 1, CHUNK], F16, tag="gath")
            nc.vector.memset(dummy[:], 0.25)
        qn = 0
        icols = CHUNK // 16  # 128
        for s in range(2):
            tabD = ttabD0 if s == 0 else ttabD1
            # per-slot idx preload: keeps the gather stream free of
            # interleaved small DMAs
            tsix = ipool.tile([128, IDXC], I16, tag="sidx", bufs=1)
            nc.sync.dma_start(out=tsix[:], in_=sidx_d[s][:])
            tdix = ipool.tile([128, IDXC], I16, tag="didx", bufs=1)
            nc.sync.dma_start(out=tdix[:], in_=didx_d[s][:])
            for c in range(NCHUNK):
                if do_gather:
                    sg = gpool.tile([128, 1, CHUNK], F16, tag="gath")
                    dg = gpool.tile([128, 1, CHUNK], F16, tag="gath")
                    nc.gpsimd.dma_gather(
                        sg[:], ttabS[:], tsix[:, c * icols:(c + 1) * icols],
                        CHUNK, CHUNK, D,
                        transpose=True, single_packet=False,
                        sbuf_tokens_per_rank=128, sbuf_free_dim_per_rank=256,
                        queue_num=qn,
                    )
                    qn = (qn + 1) % queues
                    nc.gpsimd.dma_gather(
                        dg[:], tabD[:], tdix[:, c * icols:(c + 1) * icols],
                        CHUNK, CHUNK, D,
                        transpose=True, single_packet=False,
                        sbuf_tokens_per_rank=128, sbuf_free_dim_per_rank=256,
                        queue_num=qn,
                    )
                    qn = (qn + 1) % queues
                else:
                    sg = dg = dummy
                if not do_compute:
                    continue
                for j in range(CHUNK // BLK):
                    b = s * B_SLOT + c * (CHUNK // BLK) + j
                    sT = sg[:, 0, j * BLK:(j + 1) * BLK]
                    dT = dg[:, 0, j * BLK:(j + 1) * BLK]

                    h1a = ph1.tile([128, BLK], F32, tag="ph1")
                    nc.tensor.matmul(out=h1a[:], lhsT=tw1s[:, 0:128], rhs=sT, start=True, stop=False)
                    nc.tensor.matmul(out=h1a[:], lhsT=tw1d[:, 0:128], rhs=dT, start=False, stop=True)
                    h1b = ph1.tile([128, BLK], F32, tag="ph1")
                    nc.tensor.matmul(out=h1b[:], lhsT=tw1s[:, 128:256], rhs=sT, start=True, stop=False)
                    nc.tensor.matmul(out=h1b[:], lhsT=tw1d[:, 128:256], rhs=dT, start=False, stop=True)

                    h1sa = h1pool.tile([128, BLK], F16, tag="h1s")
                    nc.scalar.activation(h1sa[:], h1a[:], AF.Relu, bias=tb1a[:])
                    h1sb = h1pool.tile([128, BLK], F16, tag="h1s")
                    nc.vector.tensor_scalar(
                        out=h1sb[:], in0=h1b[:], scalar1=tb1b[:], scalar2=0.0,
                        op0=ALU.add, op1=ALU.max,
                    )

                    h2p = ph2.tile([128, BLK], F32, tag="ph2")
                    nc.tensor.matmul(out=h2p[:], lhsT=tw2a[:], rhs=h1sa[:], start=True, stop=False)
                    nc.tensor.matmul(out=h2p[:], lhsT=tw2b[:], rhs=h1sb[:], start=False, stop=True)
                    h2s = h2pool.tile([128, BLK], F16, tag="h2s")
                    nc.vector.tensor_scalar(
                        out=h2s[:], in0=h2p[:], scalar1=tb2[:], scalar2=0.0,
                        op0=ALU.add, op1=ALU.max,
                    )

                    p, ch = b % 128, b // 128
                    if p == 0:
                        lg = plg.tile([128, BLK], F32, tag="plg")
                    nc.tensor.matmul(
                        out=lg[:], lhsT=tw3v[:, 127 - p:255 - p], rhs=h2s[:],
                        start=(p == 0), stop=(p == 127 or b == last_b),
                        skip_group_check=True,
                    )
                    if p == 127 or b == last_b:
                        nc.scalar.activation(
                            tout[:, ch * BLK:(ch + 1) * BLK], lg[:], AF.Sigmoid,
                            bias=tb3[:],
                        )

        if do_compute:
            for ch in range(OUT_CH):
                rows = min(128, B_TOT - ch * 128)
                nc.sync.dma_start(
                    out=out_d[ch * 128: ch * 128 + rows, :],
                    in_=tout[0:rows, ch * BLK:(ch + 1) * BLK],
                )


def _w3v(W3):
    v = np.zeros((128, 255), np.float16)
    v[:, 127] = W3.astype(np.float16).reshape(-1)
    return v


def _wrap_idx(arr):
    """[CAP_SLOT] int16 -> [128, IDXC] wrapped (16-partition, replicated x8)."""
    t = arr.reshape(-1, 16).T  # [16, IDXC]
    return np.tile(t, (8, 1))


def _pack_table(z16_part):
    """[<=25000, 128] fp16 -> [128, NTOK] token table (token t: partition
    t%128, rank t//128)."""
    pad = np.zeros((NTOK, D), np.float16)
    pad[: z16_part.shape[0]] = z16_part
    return np.ascontiguousarray(
        pad.reshape(NRANK, 128, D).transpose(1, 0, 2).reshape(128, NTOK)
    )


def _mlp_ref_f32(zs, zd, W1, b1, W2, b2, W3, b3):
    ef = np.concatenate([zs, zd], axis=1)
    h = np.maximum(ef @ W1 + b1, 0.0)
    h = np.maximum(h @ W2 + b2, 0.0)
    o = h @ W3 + b3
    return 1.0 / (1.0 + np.exp(-o[:, 0]))


def _pack_inputs(z, ei, W1, b1, W2, b2, W3, b3):
    """Class-bucket edges across cores; returns (in_maps, metas)."""
    src = ei[0].astype(np.int64)
    dst = ei[1].astype(np.int64)
    z16 = z.astype(np.float16)
    tables = [_pack_table(z16[r * RANGE:(r + 1) * RANGE]) for r in range(NRANGE)]
    w_common = {
        "w1s": np.ascontiguousarray(W1[:128].astype(np.float16)),
        "w1d": np.ascontiguousarray(W1[128:].astype(np.float16)),
        "w2a": np.ascontiguousarray(W2[:128].astype(np.float16)),
        "w2b": np.ascontiguousarray(W2[128:].astype(np.float16)),
        "w3v": _w3v(W3),
        "b1a": np.ascontiguousarray(b1[:128].reshape(128, 1)),
        "b1b": np.ascontiguousarray(b1[128:].reshape(128, 1)),
        "b2": np.ascontiguousarray(b2.reshape(128, 1)),
        "b3": np.full((128, 1), np.float32(b3.reshape(-1)[0])),
    }

    cls = (src // RANGE) * NRANGE + (dst // RANGE)
    order = np.argsort(cls, kind="stable")
    counts = np.bincount(cls, minlength=NCLS)
    starts = np.zeros(NCLS + 1, np.int64)
    np.cumsum(counts, out=starts[1:])

    in_maps = []
    metas = []  # per core: (positions per slot, overflow positions)
    for c in range(N_CORES):
        rs = c // 2
        rd0 = (2 * c) % NRANGE
        rd1 = rd0 + 1
        in_map = {
            **w_common,
            "tabS": tables[rs],
            "tabD0": tables[rd0],
            "tabD1": tables[rd1],
        }
        kept = []
        overflow = []
        for s in range(2):
            k = 2 * c + s
            seg = order[starts[k]:starts[k + 1]]
            if len(seg) > CAP_SLOT:
                overflow.append(seg[CAP_SLOT:])
                seg = seg[:CAP_SLOT]
            n = len(seg)
            sidx = np.zeros(CAP_SLOT, np.int16)
            didx = np.zeros(CAP_SLOT, np.int16)
            sidx[:n] = (src[seg] - rs * RANGE).astype(np.int16)
            didx[:n] = (dst[seg] - (rd0 if s == 0 else rd1) * RANGE).astype(np.int16)
            in_map[f"sidx{s}"] = np.ascontiguousarray(_wrap_idx(sidx))
            in_map[f"didx{s}"] = np.ascontiguousarray(_wrap_idx(didx))
            kept.append(seg)
        metas.append((kept, overflow))
        in_maps.append(in_map)
    return in_maps, metas


def _unpack_outputs(core_outs, metas, ei, z, W1, b1, W2, b2, W3, b3):
    E = ei.shape[1]
    out = np.empty(E, dtype=np.float32)
    for c in range(N_CORES):
        flat = np.asarray(core_outs[c], dtype=np.float32).reshape(2, CAP_SLOT)
        kept, overflow = metas[c]
        for s in range(2):
            seg = kept[s]
            out[seg] = flat[s, : len(seg)]
        for seg in overflow:
            # Host fallback for edges beyond the static per-class capacity
            # (does not trigger for the benchmark dataset).
            out[seg] = _mlp_ref_f32(
                z[ei[0, seg]], z[ei[1, seg]], W1, b1, W2, b2, W3, b3)
    return out


def _run(z, edge_index, W1, b1, W2, b2, W3, b3, **spmd_kwargs):
    global _prog_cache
    z = np.asarray(z, dtype=np.float32)
    W1 = np.asarray(W1, dtype=np.float32)
    b1 = np.asarray(b1, dtype=np.float32)
    W2 = np.asarray(W2, dtype=np.float32)
    b2 = np.asarray(b2, dtype=np.float32)
    W3 = np.asarray(W3, dtype=np.float32)
    b3 = np.asarray(b3, dtype=np.float32)
    ei = np.asarray(edge_index).astype(np.int64)
    assert z.shape == (N_NODES, D) and ei.shape[0] == 2

    if _prog_cache is None:
        _prog_cache = _build_program()
    nc = _prog_cache

    in_maps, metas = _pack_inputs(z, ei, W1, b1, W2, b2, W3, b3)
    br = run_bass_kernel_spmd(nc, in_maps, list(range(N_CORES)), **spmd_kwargs)
    core_outs = [br.results[c]["out"] for c in range(N_CORES)]
    out = _unpack_outputs(core_outs, metas, ei, z, W1, b1, W2, b2, W3, b3)
    return out, br


def kernel(z, edge_index, W1, b1, W2, b2, W3, b3):
    out, _ = _run(z, edge_index, W1, b1, W2, b2, W3, b3)
    return out
